# revision 23
# baseline (speedup 1.0000x reference)
"""GQA attention block (RoPE + causal softmax + out-projection) on 8 TRN2 cores.

Problem: q (2, 2048, 1024) 16 heads, k/v (2, 2048, 256) 4 kv heads (GQA rep 4),
causal attention, out @ w_out (1024, 1024).

Sharding: core c = (batch b = c//4, kv group = c%4). Each core computes its 4
q-heads x full T attention against its kv head, then the partial projection
X_heads @ w_out[head_rows, :]; the host sums the 4 partials per batch.

Design notes (v5): the PE clock gate (HAM) holds the tensor engine at 1.2GHz
unless it sustains near-full duty for ~3.4us, so the schedule is built to make
PE the strictly-busiest engine at 2.4GHz:
  - q groups 512 wide; S psum pool 4 deep (runahead); oT accumulators 1 bank
    per head (vaug carries 64 replicated ones-columns so the softmax
    denominator lands partition-broadcast in psum rows 64:128); projection and
    rope own the last 2 psum banks so they never steal attention slots.
  - exp split between ACT (exact) and DVE (Schraudolph int16 bit-trick into
    bf16, ~3% band that cancels in the softmax) to keep both below PE's pace.
  - causal mask = one 128-row PE matmul (ident stationary, maskT moving).
  - RoPE: x' = rotP@(x*sin) + I@(x*cos) as two accumulating PE matmuls (the
    sin/cos tables are 32-periodic so rot(x)*sin == rot(x*sin)); the two
    products come from Pool, the psum->sbuf copy from ACT; zero DVE work.
  - projection (w-chunk stationary, xT moving) is interleaved 1-2 matmuls per
    attention step via a background queue; out goes to a persistent SBUF
    staging tile, DMA'd 1024-cols at a time (DRAM-blocked layout, host
    unscrambles; partial-width DRAM APs cost ~5us of descriptor gen).
"""

import sys

if "/opt/trn_rl_repo" not in sys.path:
    sys.path.insert(0, "/opt/trn_rl_repo")

import numpy as np

B, T, D, NH, NKV, HD = 2, 2048, 1024, 16, 4, 64
HC = NH // NKV          # q heads per core = 4
CD = HC * HD            # per-core channel dim = 256
KVD = HD                # per-core kv channel dim = 64
NCORES = 8
QB = 128                # k block
GW = 512                # q group width (1 psum bank)
NGB = T // GW           # 4 groups
NKB = T // QB           # 16 k blocks
MASK = -240.0           # pre-scale additive mask; exp(-240/8) = exp(-30) ~ 1e-13
DEBUG = False

_cache: dict = {}


def _tables():
    if "tables" in _cache:
        return _cache["tables"]
    p = np.arange(128)
    t = np.arange(T)
    ang = t[None, :] / (10000.0 ** ((p[:, None] % 32) / 32.0))
    cosT = np.cos(ang).astype(np.float32)
    sinT = np.sin(ang).astype(np.float32)

    rotP = np.zeros((128, 128), np.float32)
    for base in (0, 64):
        for i in range(32):
            rotP[base + 32 + i, base + i] = -1.0   # out[i] = -x[i+32]
            rotP[base + i, base + 32 + i] = 1.0    # out[i+32] = x[i]

    kk = np.arange(QB)
    maskT = np.where(kk[:, None] <= kk[None, :], 0.0, MASK).astype(np.float32)
    ident = np.eye(128, dtype=np.float32)
    _cache["tables"] = (cosT, sinT, rotP, maskT, ident)
    return _cache["tables"]


def _build():
    import concourse.tile as tile
    from concourse import bacc, mybir

    f32 = mybir.dt.float32
    bf16 = mybir.dt.bfloat16
    i16 = mybir.dt.int16
    f8 = mybir.dt.float8e4
    Exp = mybir.ActivationFunctionType.Exp
    Mult = mybir.AluOpType.mult
    DR = mybir.MatmulPerfMode.DoubleRow
    Add = mybir.AluOpType.add
    # Schraudolph exp-as-bf16-bits: i16 = round(S*0.125*log2(e)*128 + b)
    ESH = 2.0   # common exp shift: keeps fp8 P below TRN-e4m3 max 240
    EXP_A = 0.125 * 184.6649652337873
    EXP_B = 127.0 * 128.0 - 4.6 - ESH * 184.6649652337873

    nc = bacc.Bacc("TRN2", target_bir_lowering=False, debug=False,
                   num_devices=NCORES)
    _esh = nc.alloc_sbuf_tensor("const-esh", [128, 1], f32)
    nc.gpsimd.memset(_esh.ap(), -ESH)
    nc.const_aps.aps[(f32, -ESH)] = _esh.ap()

    d_qT = nc.dram_tensor("qT", [2, 2, 128, 1024], bf16, kind="ExternalInput")
    d_kT = nc.dram_tensor("kT", [KVD, T], bf16, kind="ExternalInput")
    d_vaug = nc.dram_tensor("vaug", [128, NKB * 128], bf16,
                            kind="ExternalInput")
    d_vaug8 = nc.dram_tensor("vaug8", [128, NKB * 128], f8,
                             kind="ExternalInput")
    d_w = nc.dram_tensor("w", [CD, D], bf16, kind="ExternalInput")
    d_cosT = nc.dram_tensor("cosT", [128, T], bf16, kind="ExternalInput")
    d_sinT = nc.dram_tensor("sinT", [128, T], bf16, kind="ExternalInput")
    d_rotP = nc.dram_tensor("rotP", [128, 128], bf16, kind="ExternalInput")
    d_maskT = nc.dram_tensor("maskT", [QB, QB], bf16, kind="ExternalInput")
    d_ident = nc.dram_tensor("ident", [128, 128], bf16, kind="ExternalInput")
    d_outT = nc.dram_tensor("outT", [D // 128, 2, 128, 1024], bf16,
                            kind="ExternalOutput")
    if DEBUG:
        d_dbg_xT = nc.dram_tensor("dbg_xT", [256, T], bf16,
                                  kind="ExternalOutput")
        d_dbg_den = nc.dram_tensor("dbg_den", [1, 16 * GW], mybir.dt.float32,
                                   kind="ExternalOutput")

    with tile.TileContext(nc) as tc:
        with (
            tc.tile_pool(name="consts", bufs=1) as consts,
            tc.tile_pool(name="data", bufs=1) as data,
            tc.tile_pool(name="pt", bufs=4) as ptp,
            tc.tile_pool(name="pt8", bufs=2) as ptp8,
            tc.tile_pool(name="small", bufs=4) as small,
            tc.tile_pool(name="ropet", bufs=3) as rtp,
            tc.tile_pool(name="psS", bufs=4, space="PSUM") as psS,
            tc.tile_pool(name="psO", bufs=1, space="PSUM") as psO,
            tc.tile_pool(name="psP", bufs=2, space="PSUM") as psP,
        ):
            cosT = consts.tile([128, T], bf16)
            sinT = consts.tile([128, T], bf16)
            rotP = consts.tile([128, 128], bf16)
            maskT = consts.tile([QB, QB], bf16)
            ident = consts.tile([128, 128], bf16)
            qT = [data.tile([128, T], bf16, name=f"qT{i}", tag=f"qT{i}")
                  for i in range(2)]
            kT2 = data.tile([128, T], bf16, tag="kT2")
            vaug = data.tile([128, NKB, 128], bf16, tag="vaug")
            vaug8 = data.tile([128, NKB // 2, 2, 128], f8, tag="vaug8")
            w = [data.tile([128, D], bf16, name=f"w{i}", tag=f"w{i}")
                 for i in range(2)]
            xT = [data.tile([128, T], bf16, name=f"xT{i}", tag=f"xT{i}")
                  for i in range(2)]
            out_sb = data.tile([128, D // 128, T], bf16, tag="out_sb")
            if DEBUG:
                den_keep = data.tile([1, 16, GW], f32, tag="den_keep")

            # input DMAs spread across the 3 DMA-capable queues; rope chunk-0
            # deps land first
            nc.gpsimd.dma_start(rotP[:], d_rotP[:])
            nc.gpsimd.dma_start(qT[0][:, 0:1024], d_qT[0, 0])
            nc.gpsimd.dma_start(kT2[64:128, :], d_kT[:])
            nc.gpsimd.dma_start(maskT[:], d_maskT[:])
            nc.gpsimd.dma_start(ident[:], d_ident[:])
            nc.gpsimd.dma_start(w[0][:], d_w[0:128, :])
            nc.gpsimd.dma_start(w[1][:], d_w[128:256, :])
            nc.sync.dma_start(cosT[:], d_cosT[:])
            nc.sync.dma_start(qT[0][:, 1024:2048], d_qT[0, 1])
            nc.scalar.dma_start(sinT[:], d_sinT[:])
            nc.scalar.dma_start(kT2[0:64, :], d_kT[:])
            nc.scalar.dma_start(vaug8[:].rearrange("p n j m -> p (n j m)"),
                                d_vaug8[:])
            nc.scalar.dma_start(vaug[:].rearrange("p n m -> p (n m)"),
                                d_vaug[:])
            nc.scalar.dma_start(qT[1][:, 0:1024], d_qT[1, 0])
            nc.scalar.dma_start(qT[1][:, 1024:2048], d_qT[1, 1])

            # ---- RoPE: x' = rotP@(x*sin) + I@(x*cos), copy psum->sbuf.
            # (sin/cos are 32-periodic in d and rotate_half maps d<->d+-32,
            # so rot(x)*sin == rot(x*sin) exactly.)
            def rope_items(dst, c):
                sl = slice(512 * c, 512 * (c + 1))
                box = {}

                def i0():
                    ts = rtp.tile([128, 512], bf16, name="ts", tag="ts")
                    nc.gpsimd.tensor_mul(ts[:], dst[:, sl], sinT[:, sl])
                    box["ts"] = ts

                def i1():
                    tcs = rtp.tile([128, 512], bf16, name="tc", tag="tc")
                    nc.gpsimd.tensor_mul(tcs[:], dst[:, sl], cosT[:, sl])
                    box["tc"] = tcs

                def i2():
                    rot = psP.tile([128, 512], f32, name="rot", tag="pr")
                    nc.tensor.matmul(rot[:], rotP[:], box["ts"][:],
                                     start=True, stop=False)
                    nc.tensor.matmul(rot[:], ident[:], box["tc"][:],
                                     start=False, stop=True)
                    box["rot"] = rot

                def i3():
                    nc.scalar.copy(dst[:, sl], box["rot"][:])

                return [i0, i1, i2, i3]

            # upfront rope for group 0: qT0 chunk 0, kT2 chunk 0
            pre = [rope_items(qT[0], 0), rope_items(kT2, 0)]
            for items in pre:
                items[0]()
                items[1]()
            for items in pre:
                items[2]()
            # warmup matmuls keep PE fed while Pool/ACT finish the rope
            for i in range(16):
                wt = psS.tile([128, 128], f32, name="warm", tag="S")
                nc.tensor.matmul(wt[:], rotP[:], rotP[:], start=True,
                                 stop=True)
            for items in pre:
                items[3]()

            # background queue drained 2 items per attention step
            bg = []

            def enq(key, thunks):
                for th in thunks:
                    bg.append((key, th))

            def drain(n):
                for _ in range(min(n, len(bg))):
                    bg.pop(0)[1]()

            def drain_until(key):
                while any(k == key for k, _ in bg):
                    bg.pop(0)[1]()

            enq("q1c0", rope_items(qT[1], 0))
            enq("qkc1", rope_items(qT[0], 1))
            enq("qkc1", rope_items(kT2, 1))
            enq("q1c1", rope_items(qT[1], 1))
            enq("qkc2", rope_items(qT[0], 2))
            enq("qkc2", rope_items(kT2, 2))
            enq("q1c2", rope_items(qT[1], 2))
            enq("qkc3", rope_items(qT[0], 3))
            enq("qkc3", rope_items(kT2, 3))
            enq("q1c3", rope_items(qT[1], 3))

            need = {(0, 0): [], (0, 1): ["q1c0"],
                    (1, 0): ["qkc1"], (1, 1): ["q1c1"],
                    (2, 0): ["qkc2"], (2, 1): ["q1c2"],
                    (3, 0): ["qkc3"], (3, 1): ["q1c3"]}

            def proj_items(g):
                # projection of group g (cols qlo:qlo+512); DMA per 1024-col
                # DRAM-blocked pair after the odd group
                qlo = g * GW
                out = []
                for n in range(D // 128):
                    box = {}

                    def i0(n=n, box=box):
                        pr = psP.tile([128, GW], f32, name="pr", tag="pr")
                        for cc in range(2):
                            nc.tensor.matmul(
                                pr[:],
                                w[cc][:, n * 128:(n + 1) * 128],
                                xT[cc][:, qlo:qlo + GW],
                                start=(cc == 0), stop=(cc == 1))
                        box["pr"] = pr

                    def i1(n=n, box=box, g=g):
                        nc.vector.tensor_copy(out_sb[:, n, qlo:qlo + GW],
                                              box["pr"][:])
                        if g % 2 == 1:
                            nc.sync.dma_start(
                                d_outT[n, g // 2],
                                out_sb[:, n, qlo - GW:qlo + GW])

                    out += [i0, i1]
                return out

            # ---- attention ----
            def norm(g, hp, hh, oT):
                qlo = g * GW
                den_sb = small.tile([64, GW], f32, tag="den_sb")
                nc.scalar.copy(den_sb[:], oT[64:128, :])
                rden = small.tile([64, GW], f32, tag="rden")
                nc.vector.reciprocal_approx_fast(rden[:], den_sb[:])
                nc.vector.tensor_mul(
                    xT[hp][64 * hh:64 * hh + 64, qlo:qlo + GW],
                    oT[:HD, :], rden[:])
                if DEBUG:
                    idx = g * 4 + hp * 2 + hh
                    nc.vector.tensor_copy(den_keep[0:1, idx, :],
                                          oT[64:65, :])

            # flat step list; QK emission runs LAG steps ahead of exp/PV so
            # the PE never waits on the exp handoff or the norm chain at
            # section boundaries (QKs depend only on S slots + rope)
            steps = []
            for g in range(NGB):
                nkb = (g + 1) * (GW // QB)
                for hp in range(2):
                    for kb in range(nkb):
                        for hh in range(2):
                            steps.append((g, hp, kb, hh, nkb))
            LAG = 3
            sdict = {}
            otile = {}
            cur_qk_sec = [None]

            def emit_qk(t):
                g, hp, kb, hh, nkb = steps[t]
                if cur_qk_sec[0] != (g, hp):
                    cur_qk_sec[0] = (g, hp)
                    for key in need[(g, hp)]:
                        drain_until(key)
                qlo = g * GW
                diag = kb >= (GW // QB) * g
                cs = QB * (kb - (GW // QB) * g) if diag else 0
                qoff = 64 * hh
                S = psS.tile([128, GW], f32, name="S", tag="S")
                nc.tensor.matmul(
                    S[:, cs:], kT2[qoff:qoff + 64, kb * QB:(kb + 1) * QB],
                    qT[hp][qoff:qoff + 64, qlo + cs:qlo + GW],
                    start=True, stop=not diag, skip_group_check=True)
                if diag:
                    nc.tensor.matmul(S[:, cs:cs + QB], ident[:], maskT[:],
                                     start=False, stop=True,
                                     skip_group_check=True)
                sdict[t] = S

            pt8d = {}
            started = set()

            def emit_rest(s):
                g, hp, kb, hh, nkb = steps[s]
                diag = kb >= (GW // QB) * g
                cs = QB * (kb - (GW // QB) * g) if diag else 0
                S = sdict.pop(s)
                dve_pair = not diag and ((kb // 2) + hh) % 5 < 2
                fp8_pair = not diag and not dve_pair
                if kb == 0:
                    otile[(g, hp, hh)] = psO.tile(
                        [128, GW], f32, name=f"oT{hh}", tag=f"oT{hh}")
                oT = otile[(g, hp, hh)]

                def pv_start():
                    k = (g, hp, hh)
                    if k in started:
                        return False
                    started.add(k)
                    return True

                if fp8_pair:
                    # exp into fp8 pair tile; one DoubleRow PV per kb pair
                    if kb % 2 == 0:
                        PT8 = ptp8.tile([128, 2, GW], f8, name="PT8",
                                        tag="PT8")
                        pt8d[(g, hp, hh)] = PT8
                        nc.scalar.activation(PT8[:, 0, :], S[:], Exp,
                                             scale=0.125, bias=-ESH)
                    else:
                        PT8 = pt8d.pop((g, hp, hh))
                        nc.scalar.activation(PT8[:, 1, :], S[:], Exp,
                                             scale=0.125, bias=-ESH)
                        nc.tensor.matmul(
                            oT[:], vaug8[:, kb // 2, :, :], PT8[:, :, :],
                            start=pv_start(), stop=False,
                            perf_mode=DR, skip_group_check=True)
                else:
                    PT = ptp.tile([128, GW], bf16, name="PT", tag="PT")
                    if dve_pair:
                        nc.vector.tensor_scalar(
                            PT[:, cs:].bitcast(i16), S[:, cs:],
                            EXP_A, EXP_B, Mult, Add)
                    else:
                        nc.scalar.activation(PT[:, cs:], S[:, cs:], Exp,
                                             scale=0.125, bias=-ESH)
                    nc.tensor.matmul(oT[:, cs:], vaug[:, kb, :], PT[:, cs:],
                                     start=pv_start(), stop=(kb == nkb - 1),
                                     skip_group_check=True)
                if kb == nkb - 1:
                    norm(g, hp, hh, oT)
                    if hp == 1 and hh == 1:
                        enq(f"proj{g}", proj_items(g))

            nq = 0
            for s in range(len(steps)):
                while nq <= s + LAG and nq < len(steps):
                    emit_qk(nq)
                    nq += 1
                drain(2)
                emit_rest(s)

            while bg:
                bg.pop(0)[1]()
            if DEBUG:
                nc.sync.dma_start(d_dbg_xT[0:128, :], xT[0][:])
                nc.sync.dma_start(d_dbg_xT[128:256, :], xT[1][:])
                nc.sync.dma_start(d_dbg_den[:], den_keep[0:1, :, :])

    nc.finalize()
    return nc


def _get_nc():
    if "nc" not in _cache:
        _cache["nc"] = _build()
    return _cache["nc"]


def _in_maps(q, k, v, w_out):
    import ml_dtypes
    bf = ml_dtypes.bfloat16
    cosT, sinT, rotP, maskT, ident = _tables()
    ones = np.ones((T, 64), np.float32)
    maps = []
    for c in range(NCORES):
        b, kv = divmod(c, NKV)
        maps.append({
            "qT": np.ascontiguousarray(
                q[b, :, kv * CD:(kv + 1) * CD].T.reshape(2, 128, 2, 1024)
                .transpose(0, 2, 1, 3)).astype(bf),
            "kT": np.ascontiguousarray(k[b, :, kv * KVD:(kv + 1) * KVD].T).astype(bf),
            "vaug": np.ascontiguousarray(
                np.concatenate([v[b, :, kv * KVD:(kv + 1) * KVD], ones], 1)
                .reshape(NKB, 128, 128).transpose(1, 0, 2)
                .reshape(128, NKB * 128)).astype(bf),
            "vaug8": np.ascontiguousarray(
                np.concatenate([v[b, :, kv * KVD:(kv + 1) * KVD], ones], 1)
                .reshape(NKB, 128, 128).transpose(1, 0, 2)
                .reshape(128, NKB * 128)).astype(ml_dtypes.float8_e4m3),
            "w": np.ascontiguousarray(w_out[kv * CD:(kv + 1) * CD, :]).astype(bf),
            "cosT": cosT.astype(bf), "sinT": sinT.astype(bf),
            "rotP": rotP.astype(bf), "maskT": maskT.astype(bf),
            "ident": ident.astype(bf),
        })
    return maps


def _run(q, k, v, w_out, trace=False):
    from concourse.bass_utils import run_bass_kernel_spmd

    nc = _get_nc()
    res = run_bass_kernel_spmd(nc, _in_maps(q, k, v, w_out),
                               core_ids=list(range(NCORES)), trace=trace)
    out = np.zeros((B, T, D), np.float32)
    for c in range(NCORES):
        blk = res.results[c]["outT"]  # [8, 2, 128, 1024]
        full = blk.transpose(0, 2, 1, 3).reshape(D, T)
        out[c // NKV] += full.T.astype(np.float32)
    return out, res


def kernel(q, k, v, w_out):
    out, _ = _run(np.asarray(q), np.asarray(k), np.asarray(v),
                  np.asarray(w_out))
    return out


# revision 24
# speedup vs baseline: 1.0182x; 1.0182x over previous
"""GQA attention block (RoPE + causal softmax + out-projection) on 8 TRN2 cores.

Problem: q (2, 2048, 1024) 16 heads, k/v (2, 2048, 256) 4 kv heads (GQA rep 4),
causal attention, out @ w_out (1024, 1024).

Sharding: core c = (batch b = c//4, kv group = c%4). Each core computes its 4
q-heads x full T attention against its kv head, then the partial projection
X_heads @ w_out[head_rows, :]; the host sums the 4 partials per batch.

Design notes (v5): the PE clock gate (HAM) holds the tensor engine at 1.2GHz
unless it sustains near-full duty for ~3.4us, so the schedule is built to make
PE the strictly-busiest engine at 2.4GHz:
  - q groups 512 wide; S psum pool 4 deep (runahead); oT accumulators 1 bank
    per head (vaug carries 64 replicated ones-columns so the softmax
    denominator lands partition-broadcast in psum rows 64:128); projection and
    rope own the last 2 psum banks so they never steal attention slots.
  - exp split between ACT (exact) and DVE (Schraudolph int16 bit-trick into
    bf16, ~3% band that cancels in the softmax) to keep both below PE's pace.
  - causal mask = one 128-row PE matmul (ident stationary, maskT moving).
  - RoPE: x' = rotP@(x*sin) + I@(x*cos) as two accumulating PE matmuls (the
    sin/cos tables are 32-periodic so rot(x)*sin == rot(x*sin)); the two
    products come from Pool, the psum->sbuf copy from ACT; zero DVE work.
  - projection (w-chunk stationary, xT moving) is interleaved 1-2 matmuls per
    attention step via a background queue; out goes to a persistent SBUF
    staging tile, DMA'd 1024-cols at a time (DRAM-blocked layout, host
    unscrambles; partial-width DRAM APs cost ~5us of descriptor gen).
"""

import sys

if "/opt/trn_rl_repo" not in sys.path:
    sys.path.insert(0, "/opt/trn_rl_repo")

import numpy as np

B, T, D, NH, NKV, HD = 2, 2048, 1024, 16, 4, 64
HC = NH // NKV          # q heads per core = 4
CD = HC * HD            # per-core channel dim = 256
KVD = HD                # per-core kv channel dim = 64
NCORES = 8
QB = 128                # k block
GW = 512                # q group width (1 psum bank)
NGB = T // GW           # 4 groups
NKB = T // QB           # 16 k blocks
MASK = -240.0           # pre-scale additive mask; exp(-240/8) = exp(-30) ~ 1e-13
DEBUG = False

_cache: dict = {}


def _tables():
    if "tables" in _cache:
        return _cache["tables"]
    p = np.arange(128)
    t = np.arange(T)
    ang = t[None, :] / (10000.0 ** ((p[:, None] % 32) / 32.0))
    cosT = np.cos(ang).astype(np.float32)
    sinT = np.sin(ang).astype(np.float32)

    rotP = np.zeros((128, 128), np.float32)
    for base in (0, 64):
        for i in range(32):
            rotP[base + 32 + i, base + i] = -1.0   # out[i] = -x[i+32]
            rotP[base + i, base + 32 + i] = 1.0    # out[i+32] = x[i]

    kk = np.arange(QB)
    maskT = np.where(kk[:, None] <= kk[None, :], 0.0, MASK).astype(np.float32)
    ident = np.eye(128, dtype=np.float32)
    _cache["tables"] = (cosT, sinT, rotP, maskT, ident)
    return _cache["tables"]


def _build():
    import concourse.tile as tile
    from concourse import bacc, mybir

    f32 = mybir.dt.float32
    bf16 = mybir.dt.bfloat16
    i16 = mybir.dt.int16
    f8 = mybir.dt.float8e4
    Exp = mybir.ActivationFunctionType.Exp
    Mult = mybir.AluOpType.mult
    DR = mybir.MatmulPerfMode.DoubleRow
    Add = mybir.AluOpType.add
    # Schraudolph exp-as-bf16-bits: i16 = round(S*0.125*log2(e)*128 + b)
    ESH = 2.0   # common exp shift: keeps fp8 P below TRN-e4m3 max 240
    EXP_A = 0.125 * 184.6649652337873
    EXP_B = 127.0 * 128.0 - 4.6 - ESH * 184.6649652337873

    nc = bacc.Bacc("TRN2", target_bir_lowering=False, debug=False,
                   num_devices=NCORES)
    _esh = nc.alloc_sbuf_tensor("const-esh", [128, 1], f32)
    nc.gpsimd.memset(_esh.ap(), -ESH)
    nc.const_aps.aps[(f32, -ESH)] = _esh.ap()

    d_qT = nc.dram_tensor("qT", [2, 2, 128, 1024], bf16, kind="ExternalInput")
    d_kT = nc.dram_tensor("kT", [KVD, T], bf16, kind="ExternalInput")
    d_vaug = nc.dram_tensor("vaug", [128, NKB * 128], bf16,
                            kind="ExternalInput")
    d_vaug8 = nc.dram_tensor("vaug8", [128, NKB * 128], f8,
                             kind="ExternalInput")
    d_w = nc.dram_tensor("w", [CD, D], bf16, kind="ExternalInput")
    d_cosT = nc.dram_tensor("cosT", [128, T], bf16, kind="ExternalInput")
    d_sinT = nc.dram_tensor("sinT", [128, T], bf16, kind="ExternalInput")
    d_rotP = nc.dram_tensor("rotP", [128, 128], bf16, kind="ExternalInput")
    d_maskT = nc.dram_tensor("maskT", [QB, QB], bf16, kind="ExternalInput")
    d_ident = nc.dram_tensor("ident", [128, 128], bf16, kind="ExternalInput")
    d_outT = nc.dram_tensor("outT", [D // 128, 2, 128, 1024], bf16,
                            kind="ExternalOutput")
    if DEBUG:
        d_dbg_xT = nc.dram_tensor("dbg_xT", [256, T], bf16,
                                  kind="ExternalOutput")
        d_dbg_den = nc.dram_tensor("dbg_den", [1, 16 * GW], mybir.dt.float32,
                                   kind="ExternalOutput")

    with tile.TileContext(nc) as tc:
        with (
            tc.tile_pool(name="consts", bufs=1) as consts,
            tc.tile_pool(name="data", bufs=1) as data,
            tc.tile_pool(name="pt", bufs=4) as ptp,
            tc.tile_pool(name="pt8", bufs=2) as ptp8,
            tc.tile_pool(name="small", bufs=4) as small,
            tc.tile_pool(name="ropet", bufs=3) as rtp,
            tc.tile_pool(name="psS", bufs=4, space="PSUM") as psS,
            tc.tile_pool(name="psO", bufs=1, space="PSUM") as psO,
            tc.tile_pool(name="psP", bufs=2, space="PSUM") as psP,
        ):
            cosT = consts.tile([128, T], bf16)
            sinT = consts.tile([128, T], bf16)
            rotP = consts.tile([128, 128], bf16)
            maskT = consts.tile([QB, QB], bf16)
            ident = consts.tile([128, 128], bf16)
            qT = [data.tile([128, T], bf16, name=f"qT{i}", tag=f"qT{i}")
                  for i in range(2)]
            kT2 = data.tile([128, T], bf16, tag="kT2")
            vaug = data.tile([128, NKB, 128], bf16, tag="vaug")
            vaug8 = data.tile([128, NKB // 2, 2, 128], f8, tag="vaug8")
            w = [data.tile([128, D], bf16, name=f"w{i}", tag=f"w{i}")
                 for i in range(2)]
            xT = [data.tile([128, T], bf16, name=f"xT{i}", tag=f"xT{i}")
                  for i in range(2)]
            out_sb = data.tile([128, D // 128, T], bf16, tag="out_sb")
            if DEBUG:
                den_keep = data.tile([1, 16, GW], f32, tag="den_keep")

            # input DMAs spread across the 3 DMA-capable queues; rope chunk-0
            # deps land first
            nc.gpsimd.dma_start(rotP[:], d_rotP[:])
            nc.gpsimd.dma_start(qT[0][:, 0:1024], d_qT[0, 0])
            nc.gpsimd.dma_start(kT2[64:128, :], d_kT[:])
            nc.gpsimd.dma_start(maskT[:], d_maskT[:])
            nc.gpsimd.dma_start(ident[:], d_ident[:])
            nc.gpsimd.dma_start(w[0][:], d_w[0:128, :])
            nc.gpsimd.dma_start(w[1][:], d_w[128:256, :])
            nc.sync.dma_start(cosT[:], d_cosT[:])
            nc.sync.dma_start(qT[0][:, 1024:2048], d_qT[0, 1])
            nc.scalar.dma_start(sinT[:], d_sinT[:])
            nc.scalar.dma_start(kT2[0:64, :], d_kT[:])
            nc.scalar.dma_start(vaug8[:].rearrange("p n j m -> p (n j m)"),
                                d_vaug8[:])
            nc.scalar.dma_start(vaug[:].rearrange("p n m -> p (n m)"),
                                d_vaug[:])
            nc.scalar.dma_start(qT[1][:, 0:1024], d_qT[1, 0])
            nc.scalar.dma_start(qT[1][:, 1024:2048], d_qT[1, 1])

            # ---- RoPE: x' = rotP@(x*sin) + I@(x*cos), copy psum->sbuf.
            # (sin/cos are 32-periodic in d and rotate_half maps d<->d+-32,
            # so rot(x)*sin == rot(x*sin) exactly.)
            def rope_items(dst, c):
                sl = slice(512 * c, 512 * (c + 1))
                box = {}

                def i0():
                    ts = rtp.tile([128, 512], bf16, name="ts", tag="ts")
                    nc.gpsimd.tensor_mul(ts[:], dst[:, sl], sinT[:, sl])
                    box["ts"] = ts

                def i1():
                    tcs = rtp.tile([128, 512], bf16, name="tc", tag="tc")
                    nc.gpsimd.tensor_mul(tcs[:], dst[:, sl], cosT[:, sl])
                    box["tc"] = tcs

                def i2():
                    rot = psP.tile([128, 512], f32, name="rot", tag="pr")
                    nc.tensor.matmul(rot[:], rotP[:], box["ts"][:],
                                     start=True, stop=False)
                    nc.tensor.matmul(rot[:], ident[:], box["tc"][:],
                                     start=False, stop=True)
                    box["rot"] = rot

                def i3():
                    nc.scalar.copy(dst[:, sl], box["rot"][:])

                return [i0, i1, i2, i3]

            # upfront rope for group 0: qT0 chunk 0, kT2 chunk 0
            pre = [rope_items(qT[0], 0), rope_items(kT2, 0)]
            for items in pre:
                items[0]()
                items[1]()
            for items in pre:
                items[2]()
            # warmup matmuls keep PE fed while Pool/ACT finish the rope
            for i in range(16):
                wt = psS.tile([128, 128], f32, name="warm", tag="S")
                nc.tensor.matmul(wt[:], rotP[:], rotP[:], start=True,
                                 stop=True)
            for items in pre:
                items[3]()

            # background queue drained 2 items per attention step
            bg = []

            def enq(key, thunks):
                for th in thunks:
                    bg.append((key, th))

            def drain(n):
                for _ in range(min(n, len(bg))):
                    bg.pop(0)[1]()

            def drain_until(key):
                while any(k == key for k, _ in bg):
                    bg.pop(0)[1]()

            enq("q1c0", rope_items(qT[1], 0))
            enq("qkc1", rope_items(qT[0], 1))
            enq("qkc1", rope_items(kT2, 1))
            enq("q1c1", rope_items(qT[1], 1))
            enq("qkc2", rope_items(qT[0], 2))
            enq("qkc2", rope_items(kT2, 2))
            enq("q1c2", rope_items(qT[1], 2))
            enq("qkc3", rope_items(qT[0], 3))
            enq("qkc3", rope_items(kT2, 3))
            enq("q1c3", rope_items(qT[1], 3))

            need = {(0, 0): [], (0, 1): ["q1c0"],
                    (1, 0): ["qkc1"], (1, 1): ["q1c1"],
                    (2, 0): ["qkc2"], (2, 1): ["q1c2"],
                    (3, 0): ["qkc3"], (3, 1): ["q1c3"]}

            def proj_items(g):
                # projection of group g (cols qlo:qlo+512); DMA per 1024-col
                # DRAM-blocked pair after the odd group
                qlo = g * GW
                out = []
                for n in range(D // 128):
                    box = {}

                    def i0(n=n, box=box):
                        pr = psP.tile([128, GW], f32, name="pr", tag="pr")
                        for cc in range(2):
                            nc.tensor.matmul(
                                pr[:],
                                w[cc][:, n * 128:(n + 1) * 128],
                                xT[cc][:, qlo:qlo + GW],
                                start=(cc == 0), stop=(cc == 1))
                        box["pr"] = pr

                    def i1(n=n, box=box, g=g):
                        nc.vector.tensor_copy(out_sb[:, n, qlo:qlo + GW],
                                              box["pr"][:])
                        if g % 2 == 1:
                            nc.sync.dma_start(
                                d_outT[n, g // 2],
                                out_sb[:, n, qlo - GW:qlo + GW])

                    out += [i0, i1]
                return out

            # ---- attention ----
            def norm(g, hp, hh, oT):
                qlo = g * GW
                den_sb = small.tile([64, GW], f32, tag="den_sb")
                nc.scalar.copy(den_sb[:], oT[64:128, :])
                rden = small.tile([64, GW], f32, tag="rden")
                nc.vector.reciprocal_approx_fast(rden[:], den_sb[:])
                nc.vector.tensor_mul(
                    xT[hp][64 * hh:64 * hh + 64, qlo:qlo + GW],
                    oT[:HD, :], rden[:])
                if DEBUG:
                    idx = g * 4 + hp * 2 + hh
                    nc.vector.tensor_copy(den_keep[0:1, idx, :],
                                          oT[64:65, :])

            # flat step list; QK emission runs LAG steps ahead of exp/PV so
            # the PE never waits on the exp handoff or the norm chain at
            # section boundaries (QKs depend only on S slots + rope)
            steps = []
            for g in range(NGB):
                nkb = (g + 1) * (GW // QB)
                for hp in range(2):
                    for kb in range(nkb):
                        for hh in range(2):
                            steps.append((g, hp, kb, hh, nkb))
            LAG = 3
            sdict = {}
            otile = {}
            cur_qk_sec = [None]

            def emit_qk(t):
                g, hp, kb, hh, nkb = steps[t]
                if cur_qk_sec[0] != (g, hp):
                    cur_qk_sec[0] = (g, hp)
                    for key in need[(g, hp)]:
                        drain_until(key)
                qlo = g * GW
                diag = kb >= (GW // QB) * g
                cs = QB * (kb - (GW // QB) * g) if diag else 0
                qoff = 64 * hh
                S = psS.tile([128, GW], f32, name="S", tag="S")
                nc.tensor.matmul(
                    S[:, cs:], kT2[qoff:qoff + 64, kb * QB:(kb + 1) * QB],
                    qT[hp][qoff:qoff + 64, qlo + cs:qlo + GW],
                    start=True, stop=not diag, skip_group_check=True)
                if diag:
                    nc.tensor.matmul(S[:, cs:cs + QB], ident[:], maskT[:],
                                     start=False, stop=True,
                                     skip_group_check=True)
                sdict[t] = S

            pt8d = {}
            started = set()

            def emit_rest(s):
                g, hp, kb, hh, nkb = steps[s]
                diag = kb >= (GW // QB) * g
                cs = QB * (kb - (GW // QB) * g) if diag else 0
                S = sdict.pop(s)
                dve_pair = not diag and (2 * kb + hh) % 5 < 2
                fp8_pair = False  # fp8 DoubleRow PV measured slower here
                if kb == 0:
                    otile[(g, hp, hh)] = psO.tile(
                        [128, GW], f32, name=f"oT{hh}", tag=f"oT{hh}")
                oT = otile[(g, hp, hh)]

                def pv_start():
                    k = (g, hp, hh)
                    if k in started:
                        return False
                    started.add(k)
                    return True

                if fp8_pair:
                    # exp into fp8 pair tile; one DoubleRow PV per kb pair
                    if kb % 2 == 0:
                        PT8 = ptp8.tile([128, 2, GW], f8, name="PT8",
                                        tag="PT8")
                        pt8d[(g, hp, hh)] = PT8
                        nc.scalar.activation(PT8[:, 0, :], S[:], Exp,
                                             scale=0.125, bias=-ESH)
                    else:
                        PT8 = pt8d.pop((g, hp, hh))
                        nc.scalar.activation(PT8[:, 1, :], S[:], Exp,
                                             scale=0.125, bias=-ESH)
                        nc.tensor.matmul(
                            oT[:], vaug8[:, kb // 2, :, :], PT8[:, :, :],
                            start=pv_start(), stop=False,
                            perf_mode=DR, skip_group_check=True)
                else:
                    PT = ptp.tile([128, GW], bf16, name="PT", tag="PT")
                    if dve_pair:
                        nc.vector.tensor_scalar(
                            PT[:, cs:].bitcast(i16), S[:, cs:],
                            EXP_A, EXP_B, Mult, Add)
                    else:
                        nc.scalar.activation(PT[:, cs:], S[:, cs:], Exp,
                                             scale=0.125, bias=-ESH)
                    nc.tensor.matmul(oT[:, cs:], vaug[:, kb, :], PT[:, cs:],
                                     start=pv_start(), stop=(kb == nkb - 1),
                                     skip_group_check=True)
                if kb == nkb - 1:
                    norm(g, hp, hh, oT)
                    if hp == 1 and hh == 1:
                        enq(f"proj{g}", proj_items(g))

            nq = 0
            for s in range(len(steps)):
                while nq <= s + LAG and nq < len(steps):
                    emit_qk(nq)
                    nq += 1
                drain(2)
                emit_rest(s)

            while bg:
                bg.pop(0)[1]()
            if DEBUG:
                nc.sync.dma_start(d_dbg_xT[0:128, :], xT[0][:])
                nc.sync.dma_start(d_dbg_xT[128:256, :], xT[1][:])
                nc.sync.dma_start(d_dbg_den[:], den_keep[0:1, :, :])

    nc.finalize()
    return nc


def _get_nc():
    if "nc" not in _cache:
        _cache["nc"] = _build()
    return _cache["nc"]


def _in_maps(q, k, v, w_out):
    import ml_dtypes
    bf = ml_dtypes.bfloat16
    cosT, sinT, rotP, maskT, ident = _tables()
    ones = np.ones((T, 64), np.float32)
    maps = []
    for c in range(NCORES):
        b, kv = divmod(c, NKV)
        maps.append({
            "qT": np.ascontiguousarray(
                q[b, :, kv * CD:(kv + 1) * CD].T.reshape(2, 128, 2, 1024)
                .transpose(0, 2, 1, 3)).astype(bf),
            "kT": np.ascontiguousarray(k[b, :, kv * KVD:(kv + 1) * KVD].T).astype(bf),
            "vaug": np.ascontiguousarray(
                np.concatenate([v[b, :, kv * KVD:(kv + 1) * KVD], ones], 1)
                .reshape(NKB, 128, 128).transpose(1, 0, 2)
                .reshape(128, NKB * 128)).astype(bf),
            "vaug8": np.ascontiguousarray(
                np.concatenate([v[b, :, kv * KVD:(kv + 1) * KVD], ones], 1)
                .reshape(NKB, 128, 128).transpose(1, 0, 2)
                .reshape(128, NKB * 128)).astype(ml_dtypes.float8_e4m3),
            "w": np.ascontiguousarray(w_out[kv * CD:(kv + 1) * CD, :]).astype(bf),
            "cosT": cosT.astype(bf), "sinT": sinT.astype(bf),
            "rotP": rotP.astype(bf), "maskT": maskT.astype(bf),
            "ident": ident.astype(bf),
        })
    return maps


def _run(q, k, v, w_out, trace=False):
    from concourse.bass_utils import run_bass_kernel_spmd

    nc = _get_nc()
    res = run_bass_kernel_spmd(nc, _in_maps(q, k, v, w_out),
                               core_ids=list(range(NCORES)), trace=trace)
    out = np.zeros((B, T, D), np.float32)
    for c in range(NCORES):
        blk = res.results[c]["outT"]  # [8, 2, 128, 1024]
        full = blk.transpose(0, 2, 1, 3).reshape(D, T)
        out[c // NKV] += full.T.astype(np.float32)
    return out, res


def kernel(q, k, v, w_out):
    out, _ = _run(np.asarray(q), np.asarray(k), np.asarray(v),
                  np.asarray(w_out))
    return out
